# revision 11
# baseline (speedup 1.0000x reference)
"""Distributed LinearAndSoftmax loss kernel for 8 Trainium2 NeuronCores.

Problem: loss = mean_n[ logsumexp_v(x_n . W_v + b_v) - (x_n . W_lab_n + b_lab_n) ]
with x [16,512,768] (N=8192 rows), W [30523,768], b [30523], label [16,512].

Sharding: vocab (tensor-parallel) 8 ways -- each core computes partial
sum-exp over its 3840-column vocab shard; the label-logit dot is
data-parallel (1024 rows/core). The tiny cross-shard combine (8 x [8192]
f32 vectors) happens on host -- no on-device collective needed since the
kernel returns a scalar.

Speed design (vs the f32r baseline at ~877us; measured ~474us):
  * Matmuls run in fp8e4 (e4m3) with MatmulPerfMode.DoubleRow: each
    instruction contracts TWO 128-deep k-tiles (lhsT [128,2,128]).
  * 256-col output chunks: the DoubleRow moving operand is then 2x256 =
    512 elements, inside the ISA's 512 moving-free limit where the fp8
    double rate engages; 512-col chunks degrade to half rate.
  * kp-outer ordering shares one stationary x-pair across all psum
    chunks of a row tile, and redundant PE Ldweights instructions
    (inserted unconditionally by tile legalization, one per matmul) are
    elided post-scheduling, leaving ~6 weight loads per row tile.
  * PSUM start=True zeroes a whole 2KB bank (zero-region granularity),
    so only the first 256-col chunk of each bank carries it.
  * The vocab bias rides on DVE as a psum->SBUF add, with pad columns
    forced to exp() == 0 via a -4e6 bias. ACT then does exp(tmp * 1/SW)
    with a free-dim accumulate. The label-dot tiles interleave into the
    row loop to keep DVE work spread out.
  * W is pre-scaled by SW=1024 so fp8(W) stays in the normal range; the
    descale rides the ACT exp scale.

Accuracy: fp8 e4m3 quantization of x and W gives per-logit error
~2.5e-2 std; softmax-weighted averaging shrinks the lse error well below
1e-2, and the label logit stays in fp32 -- measured loss rel err ~2e-5
vs the fp32 reference (tolerance 2e-2).
"""

import numpy as np
import concourse.bacc as bacc
import concourse.mybir as mybir
import concourse.tile as tile
from concourse.bass_utils import run_bass_kernel_spmd

F32 = mybir.dt.float32
BF16 = mybir.dt.bfloat16
FP8 = mybir.dt.float8e4
AX = mybir.AxisListType
ALU = mybir.AluOpType
ACTF = mybir.ActivationFunctionType
DR = mybir.MatmulPerfMode.DoubleRow

B, S, D, V = 16, 512, 768, 30523
N = B * S                  # 8192 rows
NCORES = 8
VP_TOT = 30720             # padded vocab (8 * 3840)
VP = VP_TOT // NCORES      # 3840 per core
NT = N // 128              # 64 row tiles
D_PAD = 1024               # feature dim padded (pair 4 holds bias feature if used)
KT_PAD = D_PAD // 128      # 8 k-tiles
KP = 4                     # max DoubleRow k-pairs (pair 3 = bias+zeros)
LT = N // NCORES // 128    # 8 label row tiles per core
SW = 1024.0                # W/bias pre-scale so fp8(W) stays in normal range
PAD_BIAS = -4.0e6          # pre-scaled pad bias: exp(PAD_BIAS/SW) == 0 exactly

# Two PSUM tiles per row tile: banks 0-3 (2048 cols) and banks 4-7 (1792
# used). Chunks are <=512-col matmul windows, 512-aligned within the tile.
# 256-col chunks: a DoubleRow matmul then streams 2x256 = 512 moving
# elements, within the ISA's 512 moving-free limit -- the fp8 double-rate
# (2 elem/cycle) only engages there; 512-col chunks degrade to 1/cycle.
PSUM_TILES = [
    (0, 2048, [(i * 256, 256) for i in range(8)]),
    (2048, 1792, [(i * 256, 256) for i in range(7)]),
]
# Finer 4-way split: 1024-col (2-bank) PSUM tiles recycle sooner, shrinking
# the PE bubble waiting for the previous row tile's bias-add/exp consumers.
PSUM_TILES4 = [
    (0, 1024, [(i * 256, 256) for i in range(4)]),
    (1024, 1024, [(i * 256, 256) for i in range(4)]),
    (2048, 1024, [(i * 256, 256) for i in range(4)]),
    (3072, 768, [(i * 256, 256) for i in range(3)]),
]


def elide_ldweights(nc):
    """Remove back-to-back redundant PE Ldweights (same weights AP) after
    tile scheduling; tile legalization inserts one per matmul even when
    consecutive matmuls share the stationary operand. Only loads with no
    waits/updates are dropped, so synchronization is preserved."""
    for bb in nc.m.functions[0].blocks:
        insts = bb.instructions
        cur = None
        out = []
        changed = False
        for inst in insts:
            if inst.opcode == "Ldweights":
                sig = (str(inst.ins[0]), str(inst.perf_mode))
                if sig == cur and not inst.has_wait() and not inst.has_update():
                    changed = True
                    continue
                cur = sig
            out.append(inst)
        if changed:
            bb.instructions = out


def build(repeat=1, mm_only=False, pairs=3, order="kp", elide=True, bias="dve",
          psum_split=4, interleave_label=True):
    use_bias_pair = bias == "pair"
    npairs = KP if use_bias_pair else pairs
    psum_tiles = PSUM_TILES4 if psum_split == 4 else PSUM_TILES
    ptile_w = 1024 if psum_split == 4 else 2048
    nc = bacc.Bacc("TRN2", target_bir_lowering=False, debug=False, num_devices=NCORES)
    xt_d = nc.declare_dram_parameter("xt", [128, NT, KT_PAD, 128], FP8, isOutput=False)
    wt_d = nc.declare_dram_parameter("wt", [128, KT_PAD, VP], FP8, isOutput=False)
    if not use_bias_pair:
        bias_d = nc.declare_dram_parameter("biasb", [128, VP], F32, isOutput=False)
    xs_d = nc.declare_dram_parameter("xs", [128, LT, D], F32, isOutput=False)
    wl_d = nc.declare_dram_parameter("wlab", [128, LT, D], F32, isOutput=False)
    se_d = nc.declare_dram_parameter("sumexp", [128, NT], F32, isOutput=True)
    ld_d = nc.declare_dram_parameter("labdot", [128, LT], F32, isOutput=True)

    with tile.TileContext(nc) as tc:
        with (
            tc.tile_pool(name="const", bufs=1) as constp,
            tc.tile_pool(name="xtp", bufs=3) as xtp,
            tc.tile_pool(name="psum", bufs=psum_split, space="PSUM") as psum,
            tc.tile_pool(name="trp", bufs=3) as trp,
            tc.tile_pool(name="tmpp", bufs=3) as tmpp,
            tc.tile_pool(name="accp", bufs=3) as accp,
            tc.tile_pool(name="labp", bufs=2) as labp,
            tc.tile_pool(name="outp", bufs=1) as outp,
        ):
            wt = constp.tile([128, KT_PAD, VP], FP8)
            nc.sync.dma_start(wt[:], wt_d[:])
            if not use_bias_pair:
                biasb = constp.tile([128, VP], F32)
                nc.sync.dma_start(biasb[:], bias_d[:])
            se_all = outp.tile([128, NT], F32)
            ld_all = outp.tile([128, LT], F32)

            def emit_label_tile(t):
                xs_t = labp.tile([128, D], F32, tag="xs")
                nc.sync.dma_start(xs_t[:], xs_d[:, t])
                wl_t = labp.tile([128, D], F32, tag="wl")
                nc.sync.dma_start(wl_t[:], wl_d[:, t])
                tr2 = trp.tile([128, D], F32, tag="tr2")
                nc.vector.tensor_mul(tr2[:], xs_t[:], wl_t[:])
                nc.vector.tensor_reduce(
                    ld_all[:, t : t + 1], tr2[:], axis=AX.X, op=ALU.add
                )

            for _ in range(repeat):
                for t in range(NT):
                    xt_t = xtp.tile([128, KT_PAD, 128], FP8, tag="xt_t")
                    nc.sync.dma_start(xt_t[:], xt_d[:, t])
                    acc = accp.tile([128, len(psum_tiles)], F32, tag="acc")
                    pts = []
                    for g, (g0, gw, chunks) in enumerate(psum_tiles):
                        pt = psum.tile([128, ptile_w], F32, tag="pt", name=f"pt{g}")
                        pts.append((pt, g0, gw, chunks))

                    def mm(pt, g0, gw, c0, cs, kp):
                        # start=True zeroes the whole 2KB PSUM bank (2048-byte
                        # zero-region granularity), so only the first 256-col
                        # chunk of each 512-col bank may carry it; the second
                        # chunk accumulates onto the zeroed region. stop goes
                        # on the bank's final matmul (kp-outer: last chunk of
                        # the bank in the last kp pass).
                        last_in_bank = (c0 + cs == gw) or (c0 % 512 == 256)
                        nc.tensor.matmul(
                            pt[:, c0 : c0 + cs],
                            xt_t[:, 2 * kp : 2 * kp + 2, :],
                            wt[:, 2 * kp : 2 * kp + 2, g0 + c0 : g0 + c0 + cs],
                            start=(kp == 0 and c0 % 512 == 0),
                            stop=(kp == npairs - 1 and last_in_bank),
                            perf_mode=DR,
                            skip_group_check=True,
                        )

                    if order == "kp":
                        for kp in range(npairs):
                            for pt, g0, gw, chunks in pts:
                                for c0, cs in chunks:
                                    mm(pt, g0, gw, c0, cs, kp)
                    else:
                        for pt, g0, gw, chunks in pts:
                            for c0, cs in chunks:
                                for kp in range(npairs):
                                    mm(pt, g0, gw, c0, cs, kp)

                    if not mm_only:
                        for g, (pt, g0, gw, chunks) in enumerate(pts):
                            if use_bias_pair:
                                src = pt
                            else:
                                # psum -> SBUF bias add frees the PSUM tile
                                # before the (slower) exp pass reads it
                                src = tmpp.tile([128, ptile_w], F32, tag="tmp")
                                nc.vector.tensor_add(
                                    src[:, :gw], pt[:, :gw], biasb[:, g0 : g0 + gw]
                                )
                            trash = trp.tile([128, ptile_w], BF16, tag="trash")
                            nc.scalar.activation(
                                trash[:, :gw],
                                src[:, :gw],
                                ACTF.Exp,
                                scale=1.0 / SW,
                                accum_out=acc[:, g : g + 1],
                            )
                        nc.vector.tensor_reduce(
                            se_all[:, t : t + 1], acc[:], axis=AX.X, op=ALU.add
                        )

                    if interleave_label and t % (NT // LT) == 0:
                        emit_label_tile(t // (NT // LT))

                if not interleave_label:
                    for t in range(LT):
                        emit_label_tile(t)
            if mm_only:
                nc.vector.memset(se_all[:], 1.0)
                nc.vector.memset(ld_all[:], 0.0)
            nc.sync.dma_start(se_d[:], se_all[:])
            nc.sync.dma_start(ld_d[:], ld_all[:])
    if elide:
        elide_ldweights(nc)
    nc.compile()
    return nc


def prep_inputs(x, W, b, label, bias="dve"):
    """Host-side sharding + fp8 quantization: returns per-core input maps."""
    use_bias_pair = bias == "pair"
    np8 = mybir.dt.np(FP8)
    xf = np.ascontiguousarray(np.asarray(x, dtype=np.float32).reshape(N, D))
    W = np.asarray(W, dtype=np.float32)
    b = np.asarray(b, dtype=np.float32)
    lab = np.asarray(label).reshape(N).astype(np.int64)

    # x' [N, D_PAD]: features 0..767 = x, 768 = 1 (bias slot, unused when
    # the dve path is active since pair 3 is skipped), rest 0
    xp = np.zeros((N, D_PAD), dtype=np.float32)
    xp[:, :D] = xf
    xp[:, D] = 1.0
    xt = np.ascontiguousarray(
        xp.reshape(NT, 128, KT_PAD, 128).transpose(3, 0, 2, 1)
    ).astype(np8)

    # W' [VP_TOT, D_PAD]: cols 0..767 = SW*W, col 768 = SW*b (pair path), rest 0
    Wp = np.zeros((VP_TOT, D_PAD), dtype=np.float32)
    Wp[:V, :D] = SW * W
    Wp[:V, D] = SW * b

    # fp32 bias tile for the DVE path (pre-scaled; pad -> exp()==0)
    bp = np.full(VP_TOT, PAD_BIAS, dtype=np.float32)
    bp[:V] = SW * b

    in_maps = []
    for c in range(NCORES):
        Wc = Wp[c * VP : (c + 1) * VP]                      # [VP, D_PAD]
        wt = np.ascontiguousarray(
            Wc.T.reshape(KT_PAD, 128, VP).transpose(1, 0, 2)
        ).astype(np8)                                       # [128, KT_PAD, VP]
        rows = slice(c * (N // NCORES), (c + 1) * (N // NCORES))
        xs = np.ascontiguousarray(
            xf[rows].reshape(LT, 128, D).transpose(1, 0, 2)
        )
        wlab = np.ascontiguousarray(
            W[lab[rows]].reshape(LT, 128, D).transpose(1, 0, 2)
        )
        m = {"xt": xt, "wt": wt, "xs": xs, "wlab": wlab}
        if not use_bias_pair:
            m["biasb"] = np.ascontiguousarray(
                np.broadcast_to(bp[c * VP : (c + 1) * VP], (128, VP))
            )
        in_maps.append(m)
    return in_maps, lab, b


def combine(results, lab, b, bias="dve"):
    """Host-side unshard: merge per-core partials into the scalar loss."""
    sumexp = np.zeros(N, dtype=np.float64)
    labdot = np.empty(N, dtype=np.float64)
    for c in range(NCORES):
        sumexp += results[c]["sumexp"].astype(np.float64).T.reshape(N)
        rows = slice(c * (N // NCORES), (c + 1) * (N // NCORES))
        labdot[rows] = results[c]["labdot"].astype(np.float64).T.reshape(N // NCORES)
    if bias == "pair":
        sumexp -= VP_TOT - V  # each pad column contributed exactly exp(0) = 1
    lse = np.log(sumexp)
    nll = lse - (labdot + b.astype(np.float64)[lab])
    return np.asarray(nll.mean(), dtype=np.float32)


def kernel(x, W, b, label):
    in_maps, lab, b32 = prep_inputs(x, W, b, label)
    nc = build()
    res = run_bass_kernel_spmd(nc, in_maps, list(range(NCORES)), trace=False)
    return combine(res.results, lab, b32)
